# revision 7
# baseline (speedup 1.0000x reference)
"""Trainium2 Bass kernel for top-1 MoE routing (deepspeed top1gating) + expert FFN.

Strategy (8 NeuronCores, no gpsimd custom ucode — bedrock image):
  Launch 1 (token-parallel gate): core k handles tokens [k*S/8, (k+1)*S/8):
    - PE-transpose x tiles, fp32 logits = x @ w_gate on PE (fp32 keeps the
      argmax faithful to the reference),
    - per-token: argmax expert id (vector.max/max_index), top-1 softmax prob
      g = 1/sum(exp(l - max)) via ACT Exp with accum_out + DVE reciprocal,
    - casts its x slice to bf16 for the FFN launch.
  Launch 2 (expert-parallel routing positions): core k owns expert k. With
    tokens laid out [128, S/128] (s = col*128 + p), computes the reference's
    locations1 = cumsum(mask) - mask via a triangular-matrix matmul
    (within-column prefix), a DVE tensor_tensor_scan over column totals
    (across-column prefix), and a rank-1 matmul broadcast; then the capacity
    keep-mask valid = mask * (pos < C). All routing DECISIONS are on device.
  Host glue (placement only): ids[pos[valid]] = token_id[valid]; gather the
    kept rows of x_bf16 (zero row for empty slots) into the transposed
    [D, C]-layout block the matmuls want, and per-slot gate values.
  Launch 3 (expert-parallel FFN): core k owns expert k:
    - hT[f, c] = sum_d w1[d, f] xg[c, d]   (lhsT = w1 natural layout, bf16)
      gelu (tanh approx, matching jax.nn.gelu default) via x*sigmoid(2c(x+a x^3))
      out[c, d] = sum_f hT[f, c] w2[f, d]  (lhsT = hT, rhs = w2 natural, bf16)
    - scale slot rows by gate value, write compact [C, D] f32 output.
  Host scatter: out[ids[c]] = out_c[c] for filled slots; dropped and
    unrouted tokens stay zero, exactly like the reference's dense combine.
"""

import functools

import ml_dtypes
import numpy as np

import concourse.bacc as bacc
import concourse.mybir as mybir
import concourse.tile as tile
from concourse.bass_utils import run_bass_kernel_spmd

F32 = mybir.dt.float32
BF16 = mybir.dt.bfloat16
U32 = mybir.dt.uint32

N_CORES = 8

GELU_C = float(np.sqrt(2.0 / np.pi))
GELU_A = 0.044715


# --------------------------------------------------------------------------
# Launch 1: gate (token-parallel)
# --------------------------------------------------------------------------
def build_gate_nc(S, D, E, n_cores=N_CORES):
    """Per core: x_sl [S/n, D] f32 -> e_out [128, T] u32, g_out [128, T] f32,
    xb_out [S/n, D] bf16.  (T = S/n/128 tiles; token s_local = t*128 + p.)"""
    Sl = S // n_cores
    T = Sl // 128
    DC = D // 128
    nc = bacc.Bacc("TRN2", target_bir_lowering=False, debug=False)
    x_sl = nc.dram_tensor("x_sl", [Sl, D], F32, kind="ExternalInput")
    wg = nc.dram_tensor("wg", [D, E], F32, kind="ExternalInput")
    ident = nc.dram_tensor("ident", [128, 128], F32, kind="ExternalInput")
    e_out = nc.dram_tensor("e_out", [128, T], U32, kind="ExternalOutput")
    g_out = nc.dram_tensor("g_out", [128, T], F32, kind="ExternalOutput")
    xb_out = nc.dram_tensor("xb_out", [Sl, D], BF16, kind="ExternalOutput")

    with tile.TileContext(nc) as tc:
        with (
            tc.tile_pool(name="const", bufs=1) as constp,
            tc.tile_pool(name="xin", bufs=2) as xin,
            tc.tile_pool(name="xbp", bufs=2) as xbp,
            tc.tile_pool(name="xt", bufs=2) as xtp,
            tc.tile_pool(name="small", bufs=4) as small,
            tc.tile_pool(name="res", bufs=1) as resp,
            tc.tile_pool(name="pst", bufs=2, space="PSUM") as pst,
            tc.tile_pool(name="psl", bufs=2, space="PSUM") as psl,
        ):
            wg_sb = constp.tile([128, DC, E], F32)
            nc.sync.dma_start(wg_sb[:], wg.ap().rearrange("(c p) e -> p c e", p=128))
            id_sb = constp.tile([128, 128], F32)
            nc.sync.dma_start(id_sb[:], ident.ap())
            e_sb = resp.tile([128, T], U32)
            g_sb = resp.tile([128, T], F32)

            for t in range(T):
                x_sb = xin.tile([128, D], F32)
                nc.sync.dma_start(x_sb[:], x_sl.ap()[t * 128 : (t + 1) * 128, :])
                # bf16 copy of the tokens for the FFN launch
                xb = xbp.tile([128, D], BF16)
                nc.vector.tensor_copy(xb[:], x_sb[:])
                nc.sync.dma_start(xb_out.ap()[t * 128 : (t + 1) * 128, :], xb[:])
                # transpose x tile: xt_sb[:, d, :] = x_sb[:, d*128:(d+1)*128].T
                xt_sb = xtp.tile([128, DC, 128], F32)
                for d in range(DC):
                    pt = pst.tile([128, 128], F32)
                    nc.tensor.transpose(
                        pt[:], x_sb[:, d * 128 : (d + 1) * 128], id_sb[:]
                    )
                    nc.scalar.copy(xt_sb[:, d, :], pt[:])
                # logits [128 s, E] accumulated over d-chunks
                pl = psl.tile([128, E], F32)
                for d in range(DC):
                    nc.tensor.matmul(
                        pl[:],
                        lhsT=xt_sb[:, d, :],
                        rhs=wg_sb[:, d, :],
                        start=(d == 0),
                        stop=(d == DC - 1),
                    )
                lsb = small.tile([128, E], F32)
                nc.scalar.copy(lsb[:], pl[:])
                # g = 1 / sum(exp(l - max))  (softmax value at the argmax)
                mx = small.tile([128, 1], F32)
                nc.vector.tensor_reduce(
                    mx[:], lsb[:], axis=mybir.AxisListType.X, op=mybir.AluOpType.max
                )
                nmx = small.tile([128, 1], F32)
                nc.vector.tensor_scalar_mul(nmx[:], mx[:], -1.0)
                ex = small.tile([128, E], F32)
                den = small.tile([128, 1], F32)
                nc.scalar.activation(
                    ex[:],
                    lsb[:],
                    mybir.ActivationFunctionType.Exp,
                    bias=nmx[:],
                    scale=1.0,
                    accum_out=den[:],
                )
                nc.vector.reciprocal(g_sb[:, t : t + 1], den[:])
                # argmax over experts (first max wins, like jnp.argmax)
                top8 = small.tile([128, 8], F32)
                nc.vector.max(top8[:], lsb[:])
                midx = small.tile([128, 8], U32)
                nc.vector.max_index(midx[:], top8[:], lsb[:])
                nc.vector.tensor_copy(e_sb[:, t : t + 1], midx[:, 0:1])

            nc.sync.dma_start(e_out.ap()[:], e_sb[:])
            nc.sync.dma_start(g_out.ap()[:], g_sb[:])
    nc.compile()
    return nc


# --------------------------------------------------------------------------
# Launch 2: routing positions (expert-parallel)
# --------------------------------------------------------------------------
def build_route_nc(S, E):
    """Per core (expert k): exclusive position of each token within expert k
    plus the capacity keep-mask.

    Inputs : ef [128, J] f32 (expert id per token; s = j*128 + p),
             eid [128, 1] f32, ut [128, 128] f32 (ut[q, p] = 1 if q <= p).
    Outputs: pos_out [128, J] f32, valid_out [128, J] f32.
    """
    J = S // 128
    C = S // E
    nc = bacc.Bacc("TRN2", target_bir_lowering=False, debug=False)
    ef = nc.dram_tensor("ef", [128, J], F32, kind="ExternalInput")
    eid = nc.dram_tensor("eid", [128, 1], F32, kind="ExternalInput")
    ut = nc.dram_tensor("ut", [128, 128], F32, kind="ExternalInput")
    pos_out = nc.dram_tensor("pos_out", [128, J], F32, kind="ExternalOutput")
    valid_out = nc.dram_tensor("valid_out", [128, J], F32, kind="ExternalOutput")

    with tile.TileContext(nc) as tc:
        with (
            tc.tile_pool(name="p", bufs=1) as p,
            tc.tile_pool(name="ps", bufs=1, space="PSUM") as ps,
        ):
            ef_sb = p.tile([128, J], F32)
            nc.sync.dma_start(ef_sb[:], ef.ap())
            eid_sb = p.tile([128, 1], F32)
            nc.sync.dma_start(eid_sb[:], eid.ap())
            ut_sb = p.tile([128, 128], F32)
            nc.sync.dma_start(ut_sb[:], ut.ap())
            ones_sb = p.tile([1, 128], F32)
            nc.vector.memset(ones_sb[:], 1.0)
            zrow = p.tile([1, J], F32)
            nc.vector.memset(zrow[:], 0.0)

            m_sb = p.tile([128, J], F32)
            nc.vector.tensor_scalar(
                m_sb[:], ef_sb[:], eid_sb[:], None, op0=mybir.AluOpType.is_equal
            )
            # within-column inclusive prefix over partitions
            ps_incl = ps.tile([128, J], F32)
            nc.tensor.matmul(
                ps_incl[:], lhsT=ut_sb[:], rhs=m_sb[:], start=True, stop=True
            )
            incl_sb = p.tile([128, J], F32)
            nc.vector.tensor_copy(incl_sb[:], ps_incl[:])
            # column totals -> exclusive prefix across columns (tokens are
            # ordered column-major: s = j*128 + p)
            trow = p.tile([1, J], F32)
            nc.sync.dma_start(trow[:], incl_sb[127:128, :])
            sc = p.tile([1, J], F32)
            nc.vector.tensor_tensor_scan(
                sc[:],
                trow[:],
                zrow[:],
                0.0,
                op0=mybir.AluOpType.add,
                op1=mybir.AluOpType.add,
            )
            off = p.tile([1, J], F32)
            nc.vector.tensor_sub(off[:], sc[:], trow[:])
            # broadcast offsets over partitions via rank-1 matmul
            ps_off = ps.tile([128, J], F32)
            nc.tensor.matmul(
                ps_off[:], lhsT=ones_sb[:], rhs=off[:], start=True, stop=True
            )
            # exclusive position = incl - m + off
            pos_sb = p.tile([128, J], F32)
            nc.vector.tensor_sub(pos_sb[:], incl_sb[:], m_sb[:])
            nc.vector.tensor_add(pos_sb[:], pos_sb[:], ps_off[:])
            # keep-mask: member and pos < capacity
            v_sb = p.tile([128, J], F32)
            nc.vector.tensor_scalar(
                v_sb[:], pos_sb[:], float(C), None, op0=mybir.AluOpType.is_lt
            )
            nc.vector.tensor_mul(v_sb[:], v_sb[:], m_sb[:])
            nc.sync.dma_start(pos_out.ap()[:], pos_sb[:])
            nc.sync.dma_start(valid_out.ap()[:], v_sb[:])
    nc.compile()
    return nc


# --------------------------------------------------------------------------
# Launch 3: expert FFN (expert-parallel)
# --------------------------------------------------------------------------
def build_ffn_nc(S, D, E, F):
    """Per core (expert k): gathered tokens -> gelu MLP -> scaled compact out.

    Inputs : xgT [128, DC*C] bf16 (xgT[p, d*C + c] = xg[c, d*128 + p]),
             gcol [128, CC] f32 (gcol[p, cc] = gate of slot cc*128 + p),
             w1 [D, F] f32, w2 [F, D] f32.
    Outputs: out_c [C, D] f32 (slot-major, already gate-scaled).
    """
    C = S // E
    CC = C // 128
    DC = D // 128
    FC = F // 128
    NB = min(512, C)
    DB = min(512, D)
    NDB = D // DB

    nc = bacc.Bacc("TRN2", target_bir_lowering=False, debug=False)
    xgT_in = nc.dram_tensor("xgT", [128, DC * C], BF16, kind="ExternalInput")
    gcol_in = nc.dram_tensor("gcol", [128, CC], F32, kind="ExternalInput")
    w1 = nc.dram_tensor("w1", [D, F], F32, kind="ExternalInput")
    w2 = nc.dram_tensor("w2", [F, D], F32, kind="ExternalInput")
    out_c = nc.dram_tensor("out_c", [C, D], F32, kind="ExternalOutput")

    with tile.TileContext(nc) as tc:
        with (
            tc.tile_pool(name="hTp", bufs=1) as hTp,
            tc.tile_pool(name="ggp", bufs=1) as ggp,
        ):
            hT = hTp.tile([128, FC, C], BF16)
            gcol = ggp.tile([128, CC], F32)
            nc.sync.dma_start(gcol[:], gcol_in.ap())
            with tc.tile_pool(name="xgp", bufs=1) as xgp:
                xgT = xgp.tile([128, DC, C], BF16)
                nc.sync.dma_start(
                    xgT[:], xgT_in.ap().rearrange("p (a b) -> p a b", a=DC)
                )
                # ---- mm1: hT[f, c] += w1[d, f].T-chunks @ xgT, then gelu ----
                with (
                    tc.tile_pool(name="w1f32", bufs=2) as w1f32p,
                    tc.tile_pool(name="w1bf", bufs=2) as w1bfp,
                    tc.tile_pool(name="gel", bufs=2) as gel,
                    tc.tile_pool(name="ps1", bufs=2, space="PSUM") as ps1,
                ):
                    w1r = w1.ap().rearrange("(c p) f -> p c f", p=128)
                    for f in range(FC):
                        w1b = w1bfp.tile([128, DC, 128], BF16)
                        for half in range(2):
                            hc = DC // 2
                            w1f = w1f32p.tile([128, hc, 128], F32)
                            nc.sync.dma_start(
                                w1f[:],
                                w1r[
                                    :,
                                    half * hc : (half + 1) * hc,
                                    f * 128 : (f + 1) * 128,
                                ],
                            )
                            nc.vector.tensor_copy(
                                w1b[:, half * hc : (half + 1) * hc, :], w1f[:]
                            )
                        ph = ps1.tile([128, C], F32)
                        for d in range(DC):
                            for h in range(0, C, NB):
                                nc.tensor.matmul(
                                    ph[:, h : h + NB],
                                    lhsT=w1b[:, d, :],
                                    rhs=xgT[:, d, h : h + NB],
                                    start=(d == 0),
                                    stop=(d == DC - 1),
                                )
                        # gelu(tanh approx) = x * sigmoid(2c(x + a x^3))
                        x2 = gel.tile([128, C], F32)
                        nc.scalar.square(x2[:], ph[:])
                        nc.vector.tensor_scalar(
                            x2[:],
                            x2[:],
                            2.0 * GELU_C * GELU_A,
                            2.0 * GELU_C,
                            op0=mybir.AluOpType.mult,
                            op1=mybir.AluOpType.add,
                        )
                        nc.vector.tensor_mul(x2[:], x2[:], ph[:])
                        sg = gel.tile([128, C], F32)
                        nc.scalar.activation(
                            sg[:], x2[:], mybir.ActivationFunctionType.Sigmoid
                        )
                        nc.vector.tensor_mul(hT[:, f, :], sg[:], ph[:])

            # ---- mm2: out[c, d] += hT[f, c].T @ w2[f, d], scaled by gate ----
            with (
                tc.tile_pool(name="w2f32", bufs=2) as w2f32p,
                tc.tile_pool(name="w2bf", bufs=2) as w2bfp,
                tc.tile_pool(name="outp", bufs=2) as outp,
                tc.tile_pool(name="ps2", bufs=1, space="PSUM") as ps2,
            ):
                for db in range(NDB):
                    pso = [
                        ps2.tile([128, DB], F32, name=f"pso{c}", tag=f"pso{c}")
                        for c in range(CC)
                    ]
                    for f in range(FC):
                        w2f = w2f32p.tile([128, DB], F32)
                        nc.sync.dma_start(
                            w2f[:],
                            w2.ap()[
                                f * 128 : (f + 1) * 128, db * DB : (db + 1) * DB
                            ],
                        )
                        w2b = w2bfp.tile([128, DB], BF16)
                        nc.vector.tensor_copy(w2b[:], w2f[:])
                        for c in range(CC):
                            nc.tensor.matmul(
                                pso[c][:],
                                lhsT=hT[:, f, c * 128 : (c + 1) * 128],
                                rhs=w2b[:],
                                start=(f == 0),
                                stop=(f == FC - 1),
                            )
                    for c in range(CC):
                        ob = outp.tile([128, DB], F32)
                        nc.vector.tensor_scalar(
                            ob[:],
                            pso[c][:],
                            gcol[:, c : c + 1],
                            None,
                            op0=mybir.AluOpType.mult,
                        )
                        nc.sync.dma_start(
                            out_c.ap()[
                                c * 128 : (c + 1) * 128, db * DB : (db + 1) * DB
                            ],
                            ob[:],
                        )
    nc.compile()
    return nc


# --------------------------------------------------------------------------
# Host orchestration
# --------------------------------------------------------------------------
@functools.lru_cache(maxsize=None)
def _compiled(S, D, E, F, n_cores):
    return (
        build_gate_nc(S, D, E, n_cores),
        build_route_nc(S, E),
        build_ffn_nc(S, D, E, F),
    )


def _run_spmd(nc, in_maps, **kw):
    res = run_bass_kernel_spmd(nc, in_maps, core_ids=list(range(len(in_maps))), **kw)
    return res.results


def moe_forward(hidden_states, w_gate, w1, w2, n_cores=N_CORES, run=_run_spmd):
    B, T, D = hidden_states.shape
    E = w_gate.shape[1]
    F = w1.shape[2]
    S = B * T
    C = S // E
    CC = C // 128
    DC = D // 128
    Sl = S // n_cores
    J = S // 128
    x = np.ascontiguousarray(hidden_states.reshape(S, D), dtype=np.float32)
    nc_gate, nc_route, nc_ffn = _compiled(S, D, E, F, n_cores)

    # ---- launch 1: gate ----
    ident = np.eye(128, dtype=np.float32)
    wg = np.ascontiguousarray(w_gate, dtype=np.float32)
    in1 = [
        {"x_sl": x[k * Sl : (k + 1) * Sl], "wg": wg, "ident": ident}
        for k in range(n_cores)
    ]
    res1 = run(nc_gate, in1)

    # per-core outputs concat: column j = (k, t) -> token s = j*128 + p
    ef = np.concatenate([r["e_out"] for r in res1], axis=1).astype(np.float32)
    gf = np.concatenate([r["g_out"] for r in res1], axis=1)  # [128, J] f32
    xb_full = np.concatenate(
        [r["xb_out"] for r in res1] + [np.zeros((1, D), dtype=ml_dtypes.bfloat16)]
    )  # [S+1, D]

    # ---- launch 2: routing positions ----
    ut = np.tril(np.ones((128, 128), dtype=np.float32)).T  # ut[q, p] = q <= p
    in2 = [
        {"ef": ef, "eid": np.full((128, 1), float(k), np.float32), "ut": ut}
        for k in range(n_cores)
    ]
    res2 = run(nc_route, in2)

    # ---- host glue: build per-expert slot -> token index lists (placement) --
    s_val = (np.arange(J)[None, :] * 128 + np.arange(128)[:, None]).astype(
        np.int64
    )  # [128, J]
    g_vec = np.empty(S, dtype=np.float32)
    g_vec[s_val.reshape(-1)] = gf.reshape(-1)
    ids_all = []
    in3 = []
    for k in range(n_cores):
        pos = res2[k]["pos_out"]
        valid = res2[k]["valid_out"] > 0.5
        ids = np.full(C, S, dtype=np.int64)  # default -> zero row
        ids[pos[valid].astype(np.int64)] = s_val[valid]
        ids_all.append(ids)
        xg = xb_full[ids]  # [C, D] bf16
        xgT = np.ascontiguousarray(
            xg.T.reshape(DC, 128, C).transpose(1, 0, 2).reshape(128, DC * C)
        )
        g_slot = np.where(ids < S, g_vec[np.minimum(ids, S - 1)], 0.0).astype(
            np.float32
        )
        gcol = np.ascontiguousarray(g_slot.reshape(CC, 128).T)
        in3.append(
            {
                "xgT": xgT,
                "gcol": gcol,
                "w1": np.ascontiguousarray(w1[k], dtype=np.float32),
                "w2": np.ascontiguousarray(w2[k], dtype=np.float32),
            }
        )

    # ---- launch 3: FFN ----
    res3 = run(nc_ffn, in3)

    # ---- host scatter (placement only) ----
    out = np.zeros((S, D), dtype=np.float32)
    for k in range(n_cores):
        ids = ids_all[k]
        filled = ids < S
        out[ids[filled]] = res3[k]["out_c"][filled]
    return out.reshape(B, T, D)


def kernel(**inputs):
    hs = np.asarray(inputs["hidden_states"], dtype=np.float32)
    wg = np.asarray(inputs["w_gate"], dtype=np.float32)
    w1 = np.asarray(inputs["w1"], dtype=np.float32)
    w2 = np.asarray(inputs["w2"], dtype=np.float32)
    return moe_forward(hs, wg, w1, w2)


# revision 9
# speedup vs baseline: 2.1804x; 2.1804x over previous
"""Trainium2 Bass kernel for top-1 MoE routing (deepspeed top1gating) + expert FFN.

Strategy (8 NeuronCores, no gpsimd custom ucode — bedrock image):
  Launch 1 (token-parallel gate): core k handles tokens [k*S/8, (k+1)*S/8):
    - PE-transpose x tiles, fp32 logits = x @ w_gate on PE (fp32 keeps the
      argmax faithful to the reference),
    - per-token: argmax expert id (vector.max/max_index), top-1 softmax prob
      g = 1/sum(exp(l - max)) via ACT Exp with accum_out + DVE reciprocal,
    - casts its x slice to bf16 for the FFN launch.
  Launch 2 (expert-parallel routing positions): core k owns expert k. With
    tokens laid out [128, S/128] (s = col*128 + p), computes the reference's
    locations1 = cumsum(mask) - mask via a triangular-matrix matmul
    (within-column prefix), a DVE tensor_tensor_scan over column totals
    (across-column prefix), and a rank-1 matmul broadcast; then the capacity
    keep-mask valid = mask * (pos < C). All routing DECISIONS are on device.
  Host glue (placement only): ids[pos[valid]] = token_id[valid]; gather the
    kept rows of x_bf16 (zero row for empty slots) into the transposed
    [D, C]-layout block the matmuls want, and per-slot gate values.
  Launch 3 (expert-parallel FFN): core k owns expert k:
    - hT[f, c] = sum_d w1[d, f] xg[c, d]   (lhsT = w1 natural layout, bf16)
      gelu (tanh approx, matching jax.nn.gelu default) via x*sigmoid(2c(x+a x^3))
      out[c, d] = sum_f hT[f, c] w2[f, d]  (lhsT = hT, rhs = w2 natural, bf16)
    - scale slot rows by gate value, write compact [C, D] f32 output.
  Host scatter: out[ids[c]] = out_c[c] for filled slots; dropped and
    unrouted tokens stay zero, exactly like the reference's dense combine.
"""

import functools

import ml_dtypes
import numpy as np

import concourse.bacc as bacc
import concourse.mybir as mybir
import concourse.tile as tile
from concourse.bass_utils import run_bass_kernel_spmd

F32 = mybir.dt.float32
BF16 = mybir.dt.bfloat16
U32 = mybir.dt.uint32

N_CORES = 8

GELU_C = float(np.sqrt(2.0 / np.pi))
GELU_A = 0.044715


# --------------------------------------------------------------------------
# Launch 1: gate (token-parallel)
# --------------------------------------------------------------------------
def build_gate_nc(S, D, E, n_cores=N_CORES, reps=1):
    """Per core: x_sl [S/n, D] f32 -> e_out [128, T] u32, g_out [128, T] f32,
    xb_out [S/n, D] bf16.  (T = S/n/128 tiles; token s_local = t*128 + p.)"""
    Sl = S // n_cores
    T = Sl // 128
    DC = D // 128
    nc = bacc.Bacc("TRN2", target_bir_lowering=False, debug=False)
    x_sl = nc.dram_tensor("x_sl", [Sl, D], F32, kind="ExternalInput")
    wg = nc.dram_tensor("wg", [D, E], F32, kind="ExternalInput")
    ident = nc.dram_tensor("ident", [128, 128], F32, kind="ExternalInput")
    e_out = nc.dram_tensor("e_out", [128, T], U32, kind="ExternalOutput")
    g_out = nc.dram_tensor("g_out", [128, T], F32, kind="ExternalOutput")
    xb_out = nc.dram_tensor("xb_out", [Sl, D], BF16, kind="ExternalOutput")

    with tile.TileContext(nc) as tc:
        with (
            tc.tile_pool(name="const", bufs=1) as constp,
            tc.tile_pool(name="xin", bufs=2) as xin,
            tc.tile_pool(name="xbp", bufs=2) as xbp,
            tc.tile_pool(name="xt", bufs=2) as xtp,
            tc.tile_pool(name="small", bufs=4) as small,
            tc.tile_pool(name="res", bufs=1) as resp,
            tc.tile_pool(name="pst", bufs=2, space="PSUM") as pst,
            tc.tile_pool(name="psl", bufs=2, space="PSUM") as psl,
        ):
            wg_sb = constp.tile([128, DC, E], F32)
            nc.sync.dma_start(wg_sb[:], wg.ap().rearrange("(c p) e -> p c e", p=128))
            id_sb = constp.tile([128, 128], F32)
            nc.sync.dma_start(id_sb[:], ident.ap())
            e_sb = resp.tile([128, T], U32)
            g_sb = resp.tile([128, T], F32)

            for t in [t for _ in range(reps) for t in range(T)]:
                x_sb = xin.tile([128, D], F32)
                nc.sync.dma_start(x_sb[:], x_sl.ap()[t * 128 : (t + 1) * 128, :])
                # bf16 copy of the tokens for the FFN launch
                xb = xbp.tile([128, D], BF16)
                nc.vector.tensor_copy(xb[:], x_sb[:])
                nc.sync.dma_start(xb_out.ap()[t * 128 : (t + 1) * 128, :], xb[:])
                # transpose x tile: xt_sb[:, d, :] = x_sb[:, d*128:(d+1)*128].T
                xt_sb = xtp.tile([128, DC, 128], F32)
                for d in range(DC):
                    pt = pst.tile([128, 128], F32)
                    nc.tensor.transpose(
                        pt[:], x_sb[:, d * 128 : (d + 1) * 128], id_sb[:]
                    )
                    nc.scalar.copy(xt_sb[:, d, :], pt[:])
                # logits [128 s, E] accumulated over d-chunks
                pl = psl.tile([128, E], F32)
                for d in range(DC):
                    nc.tensor.matmul(
                        pl[:],
                        lhsT=xt_sb[:, d, :],
                        rhs=wg_sb[:, d, :],
                        start=(d == 0),
                        stop=(d == DC - 1),
                    )
                lsb = small.tile([128, E], F32)
                nc.scalar.copy(lsb[:], pl[:])
                # g = 1 / sum(exp(l - max))  (softmax value at the argmax)
                mx = small.tile([128, 1], F32)
                nc.vector.tensor_reduce(
                    mx[:], lsb[:], axis=mybir.AxisListType.X, op=mybir.AluOpType.max
                )
                nmx = small.tile([128, 1], F32)
                nc.vector.tensor_scalar_mul(nmx[:], mx[:], -1.0)
                ex = small.tile([128, E], F32)
                den = small.tile([128, 1], F32)
                nc.scalar.activation(
                    ex[:],
                    lsb[:],
                    mybir.ActivationFunctionType.Exp,
                    bias=nmx[:],
                    scale=1.0,
                    accum_out=den[:],
                )
                nc.vector.reciprocal(g_sb[:, t : t + 1], den[:])
                # argmax over experts (first max wins, like jnp.argmax)
                top8 = small.tile([128, 8], F32)
                nc.vector.max(top8[:], lsb[:])
                midx = small.tile([128, 8], U32)
                nc.vector.max_index(midx[:], top8[:], lsb[:])
                nc.vector.tensor_copy(e_sb[:, t : t + 1], midx[:, 0:1])

            nc.sync.dma_start(e_out.ap()[:], e_sb[:])
            nc.sync.dma_start(g_out.ap()[:], g_sb[:])
    nc.compile()
    return nc


# --------------------------------------------------------------------------
# Launch 2: routing positions (expert-parallel)
# --------------------------------------------------------------------------
def build_route_nc(S, E, reps=1):
    """Per core (expert k): exclusive position of each token within expert k
    plus the capacity keep-mask.

    Inputs : ef [128, J] f32 (expert id per token; s = j*128 + p),
             eid [128, 1] f32, ut [128, 128] f32 (ut[q, p] = 1 if q <= p).
    Outputs: pos_out [128, J] f32, valid_out [128, J] f32.
    """
    J = S // 128
    C = S // E
    nc = bacc.Bacc("TRN2", target_bir_lowering=False, debug=False)
    ef = nc.dram_tensor("ef", [128, J], F32, kind="ExternalInput")
    eid = nc.dram_tensor("eid", [128, 1], F32, kind="ExternalInput")
    ut = nc.dram_tensor("ut", [128, 128], F32, kind="ExternalInput")
    pos_out = nc.dram_tensor("pos_out", [128, J], F32, kind="ExternalOutput")
    valid_out = nc.dram_tensor("valid_out", [128, J], F32, kind="ExternalOutput")

    with tile.TileContext(nc) as tc:
        for _rep in range(reps):
          with (
            tc.tile_pool(name="p", bufs=1) as p,
            tc.tile_pool(name="ps", bufs=1, space="PSUM") as ps,
          ):
            ef_sb = p.tile([128, J], F32)
            nc.sync.dma_start(ef_sb[:], ef.ap())
            eid_sb = p.tile([128, 1], F32)
            nc.sync.dma_start(eid_sb[:], eid.ap())
            ut_sb = p.tile([128, 128], F32)
            nc.sync.dma_start(ut_sb[:], ut.ap())
            ones_sb = p.tile([1, 128], F32)
            nc.vector.memset(ones_sb[:], 1.0)
            zrow = p.tile([1, J], F32)
            nc.vector.memset(zrow[:], 0.0)

            m_sb = p.tile([128, J], F32)
            nc.vector.tensor_scalar(
                m_sb[:], ef_sb[:], eid_sb[:], None, op0=mybir.AluOpType.is_equal
            )
            # within-column inclusive prefix over partitions
            ps_incl = ps.tile([128, J], F32)
            nc.tensor.matmul(
                ps_incl[:], lhsT=ut_sb[:], rhs=m_sb[:], start=True, stop=True
            )
            incl_sb = p.tile([128, J], F32)
            nc.vector.tensor_copy(incl_sb[:], ps_incl[:])
            # column totals -> exclusive prefix across columns (tokens are
            # ordered column-major: s = j*128 + p)
            trow = p.tile([1, J], F32)
            nc.sync.dma_start(trow[:], incl_sb[127:128, :])
            sc = p.tile([1, J], F32)
            nc.vector.tensor_tensor_scan(
                sc[:],
                trow[:],
                zrow[:],
                0.0,
                op0=mybir.AluOpType.add,
                op1=mybir.AluOpType.add,
            )
            off = p.tile([1, J], F32)
            nc.vector.tensor_sub(off[:], sc[:], trow[:])
            # broadcast offsets over partitions via rank-1 matmul
            ps_off = ps.tile([128, J], F32)
            nc.tensor.matmul(
                ps_off[:], lhsT=ones_sb[:], rhs=off[:], start=True, stop=True
            )
            # exclusive position = incl - m + off
            pos_sb = p.tile([128, J], F32)
            nc.vector.tensor_sub(pos_sb[:], incl_sb[:], m_sb[:])
            nc.vector.tensor_add(pos_sb[:], pos_sb[:], ps_off[:])
            # keep-mask: member and pos < capacity
            v_sb = p.tile([128, J], F32)
            nc.vector.tensor_scalar(
                v_sb[:], pos_sb[:], float(C), None, op0=mybir.AluOpType.is_lt
            )
            nc.vector.tensor_mul(v_sb[:], v_sb[:], m_sb[:])
            nc.sync.dma_start(pos_out.ap()[:], pos_sb[:])
            nc.sync.dma_start(valid_out.ap()[:], v_sb[:])
    nc.compile()
    return nc


# --------------------------------------------------------------------------
# Launch 3: expert FFN (expert-parallel)
# --------------------------------------------------------------------------
def build_ffn_nc(S, D, E, F, reps=1):
    """Per core (expert k): gathered tokens -> gelu MLP -> scaled compact out.

    Inputs : xgT [128, DC*C] bf16 (xgT[p, d*C + c] = xg[c, d*128 + p]),
             gcol [128, CC] f32 (gcol[p, cc] = gate of slot cc*128 + p),
             w1 [D, F] f32, w2 [F, D] f32.
    Outputs: out_c [C, D] f32 (slot-major, already gate-scaled).
    """
    C = S // E
    CC = C // 128
    DC = D // 128
    FC = F // 128
    NB = min(512, C)
    DB = min(512, D)
    NDB = D // DB

    nc = bacc.Bacc("TRN2", target_bir_lowering=False, debug=False)
    xgT_in = nc.dram_tensor("xgT", [128, DC * C], BF16, kind="ExternalInput")
    gcol_in = nc.dram_tensor("gcol", [128, CC], F32, kind="ExternalInput")
    w1 = nc.dram_tensor("w1", [D, F], F32, kind="ExternalInput")
    w2 = nc.dram_tensor("w2", [F, D], F32, kind="ExternalInput")
    out_c = nc.dram_tensor("out_c", [C, D], F32, kind="ExternalOutput")

    with tile.TileContext(nc) as tc:
      for _rep in range(reps):
        with (
            tc.tile_pool(name="hTp", bufs=1) as hTp,
            tc.tile_pool(name="ggp", bufs=1) as ggp,
        ):
            hT = hTp.tile([128, FC, C], BF16)
            gcol = ggp.tile([128, CC], F32)
            nc.sync.dma_start(gcol[:], gcol_in.ap())
            with tc.tile_pool(name="xgp", bufs=1) as xgp:
                xgT = xgp.tile([128, DC, C], BF16)
                nc.sync.dma_start(
                    xgT[:], xgT_in.ap().rearrange("p (a b) -> p a b", a=DC)
                )
                # ---- mm1: hT[f, c] += w1[d, f].T-chunks @ xgT, then gelu ----
                with (
                    tc.tile_pool(name="w1f32", bufs=3) as w1f32p,
                    tc.tile_pool(name="w1bf", bufs=3) as w1bfp,
                    tc.tile_pool(name="gel", bufs=2) as gel,
                    tc.tile_pool(name="ps1", bufs=2, space="PSUM") as ps1,
                ):
                    w1r = w1.ap().rearrange("(c p) f -> p c f", p=128)
                    for f in range(FC):
                        w1b = w1bfp.tile([128, DC, 128], BF16)
                        for half in range(2):
                            hc = DC // 2
                            w1f = w1f32p.tile([128, hc, 128], F32)
                            nc.sync.dma_start(
                                w1f[:],
                                w1r[
                                    :,
                                    half * hc : (half + 1) * hc,
                                    f * 128 : (f + 1) * 128,
                                ],
                            )
                            nc.vector.tensor_copy(
                                w1b[:, half * hc : (half + 1) * hc, :], w1f[:]
                            )
                        ph = ps1.tile([128, C], F32)
                        for d in range(DC):
                            for h in range(0, C, NB):
                                nc.tensor.matmul(
                                    ph[:, h : h + NB],
                                    lhsT=w1b[:, d, :],
                                    rhs=xgT[:, d, h : h + NB],
                                    start=(d == 0),
                                    stop=(d == DC - 1),
                                )
                        # gelu(tanh approx) = x * sigmoid(2c(x + a x^3))
                        x2 = gel.tile([128, C], F32)
                        nc.scalar.square(x2[:], ph[:])
                        nc.vector.tensor_scalar(
                            x2[:],
                            x2[:],
                            2.0 * GELU_C * GELU_A,
                            2.0 * GELU_C,
                            op0=mybir.AluOpType.mult,
                            op1=mybir.AluOpType.add,
                        )
                        nc.vector.tensor_mul(x2[:], x2[:], ph[:])
                        sg = gel.tile([128, C], F32)
                        nc.scalar.activation(
                            sg[:], x2[:], mybir.ActivationFunctionType.Sigmoid
                        )
                        nc.vector.tensor_mul(hT[:, f, :], sg[:], ph[:])

            # ---- mm2: out[c, d] += hT[f, c].T @ w2[f, d], scaled by gate ----
            with (
                tc.tile_pool(name="w2f32", bufs=3) as w2f32p,
                tc.tile_pool(name="w2bf", bufs=3) as w2bfp,
                tc.tile_pool(name="outp", bufs=4) as outp,
                tc.tile_pool(name="ps2", bufs=1, space="PSUM") as ps2,
            ):
                for db in range(NDB):
                    pso = [
                        ps2.tile([128, DB], F32, name=f"pso{c}", tag=f"pso{c}")
                        for c in range(CC)
                    ]
                    for f in range(FC):
                        w2f = w2f32p.tile([128, DB], F32)
                        nc.sync.dma_start(
                            w2f[:],
                            w2.ap()[
                                f * 128 : (f + 1) * 128, db * DB : (db + 1) * DB
                            ],
                        )
                        w2b = w2bfp.tile([128, DB], BF16)
                        nc.vector.tensor_copy(w2b[:], w2f[:])
                        for c in range(CC):
                            nc.tensor.matmul(
                                pso[c][:],
                                lhsT=hT[:, f, c * 128 : (c + 1) * 128],
                                rhs=w2b[:],
                                start=(f == 0),
                                stop=(f == FC - 1),
                            )
                    for c in range(CC):
                        ob = outp.tile([128, DB], F32)
                        nc.vector.tensor_scalar(
                            ob[:],
                            pso[c][:],
                            gcol[:, c : c + 1],
                            None,
                            op0=mybir.AluOpType.mult,
                        )
                        nc.sync.dma_start(
                            out_c.ap()[
                                c * 128 : (c + 1) * 128, db * DB : (db + 1) * DB
                            ],
                            ob[:],
                        )
    nc.compile()
    return nc


# --------------------------------------------------------------------------
# Host orchestration
# --------------------------------------------------------------------------
@functools.lru_cache(maxsize=None)
def _compiled(S, D, E, F, n_cores):
    return (
        build_gate_nc(S, D, E, n_cores),
        build_route_nc(S, E),
        build_ffn_nc(S, D, E, F),
    )


def _run_spmd(nc, in_maps, **kw):
    res = run_bass_kernel_spmd(nc, in_maps, core_ids=list(range(len(in_maps))), **kw)
    return res.results


def moe_forward(hidden_states, w_gate, w1, w2, n_cores=N_CORES, run=_run_spmd):
    B, T, D = hidden_states.shape
    E = w_gate.shape[1]
    F = w1.shape[2]
    S = B * T
    C = S // E
    CC = C // 128
    DC = D // 128
    Sl = S // n_cores
    J = S // 128
    x = np.ascontiguousarray(hidden_states.reshape(S, D), dtype=np.float32)
    nc_gate, nc_route, nc_ffn = _compiled(S, D, E, F, n_cores)

    # ---- launch 1: gate ----
    ident = np.eye(128, dtype=np.float32)
    wg = np.ascontiguousarray(w_gate, dtype=np.float32)
    in1 = [
        {"x_sl": x[k * Sl : (k + 1) * Sl], "wg": wg, "ident": ident}
        for k in range(n_cores)
    ]
    res1 = run(nc_gate, in1)

    # per-core outputs concat: column j = (k, t) -> token s = j*128 + p
    ef = np.concatenate([r["e_out"] for r in res1], axis=1).astype(np.float32)
    gf = np.concatenate([r["g_out"] for r in res1], axis=1)  # [128, J] f32
    xb_full = np.concatenate(
        [r["xb_out"] for r in res1] + [np.zeros((1, D), dtype=ml_dtypes.bfloat16)]
    )  # [S+1, D]

    # ---- launch 2: routing positions ----
    ut = np.tril(np.ones((128, 128), dtype=np.float32)).T  # ut[q, p] = q <= p
    in2 = [
        {"ef": ef, "eid": np.full((128, 1), float(k), np.float32), "ut": ut}
        for k in range(n_cores)
    ]
    res2 = run(nc_route, in2)

    # ---- host glue: build per-expert slot -> token index lists (placement) --
    s_val = (np.arange(J)[None, :] * 128 + np.arange(128)[:, None]).astype(
        np.int64
    )  # [128, J]
    g_vec = np.empty(S, dtype=np.float32)
    g_vec[s_val.reshape(-1)] = gf.reshape(-1)
    ids_all = []
    in3 = []
    for k in range(n_cores):
        pos = res2[k]["pos_out"]
        valid = res2[k]["valid_out"] > 0.5
        ids = np.full(C, S, dtype=np.int64)  # default -> zero row
        ids[pos[valid].astype(np.int64)] = s_val[valid]
        ids_all.append(ids)
        xg = xb_full[ids]  # [C, D] bf16
        xgT = np.ascontiguousarray(
            xg.T.reshape(DC, 128, C).transpose(1, 0, 2).reshape(128, DC * C)
        )
        g_slot = np.where(ids < S, g_vec[np.minimum(ids, S - 1)], 0.0).astype(
            np.float32
        )
        gcol = np.ascontiguousarray(g_slot.reshape(CC, 128).T)
        in3.append(
            {
                "xgT": xgT,
                "gcol": gcol,
                "w1": np.ascontiguousarray(w1[k], dtype=np.float32),
                "w2": np.ascontiguousarray(w2[k], dtype=np.float32),
            }
        )

    # ---- launch 3: FFN ----
    res3 = run(nc_ffn, in3)

    # ---- host scatter (placement only) ----
    out = np.zeros((S, D), dtype=np.float32)
    for k in range(n_cores):
        ids = ids_all[k]
        filled = ids < S
        out[ids[filled]] = res3[k]["out_c"][filled]
    return out.reshape(B, T, D)


def kernel(**inputs):
    hs = np.asarray(inputs["hidden_states"], dtype=np.float32)
    wg = np.asarray(inputs["w_gate"], dtype=np.float32)
    w1 = np.asarray(inputs["w1"], dtype=np.float32)
    w2 = np.asarray(inputs["w2"], dtype=np.float32)
    return moe_forward(hs, wg, w1, w2)
